# revision 35
# baseline (speedup 1.0000x reference)
"""Trainium2 Bass kernel for the ExemplarBaseline retrieval-kNN model.

Math (per batch b, fully independent across b):
    f      = data.reshape(B*T, CHW) @ W_fe + b_fe            (feature extract)
    d2     = ||f_s - f_t||^2 ; dist = d2**0.25
    sims   = exp(-c * dist)
    numers = 1e-8 + sum_{s<t} sims[s,t] * teach[s, cls]
    score  = numers**gamma / sum_cls ; score[t=0] = 1e-8

Sharding: data-parallel over the batch dim B (128) across 8 NeuronCores,
16 sequences per core.  Host pre-casts x/W to fp8 and pre-transposes x so
the device only does matmuls + a fused epilogue:

  - feats^T [D, tok] = W^T @ x^T, fp8 DoubleRow MMs (24 K-tiles -> 12 MMs)
  - evac psum->fT (bf16) on DVE with bias add; f2 = Square(psum+bias) on
    ACT (Square lives in the same ACT table set as Ln/Exp -> no reloads)
  - sq[tok] = ones^T @ f2 on PE; sqn = -0.5*sq as a plain bf16 row
  - per 4-seq chunk: pg[:,bi,:] = Gram (8 bf16 MMs) + 2 rank-1 MMs
    (sqn x ones, ones x sqn) adding -0.5*(sq_s+sq_t) => pg = -0.5*d2
  - batched epilogue on the whole [128, 4*128] chunk:
      d2->dist->sims via Ln/Exp/Exp on ACT, causal mask via ONE
      affine_select on GpSimd, numers = ONE bf16 MM per seq,
      gamma==1 fast path (DVE only): score = pn*rden + eps*rden with
      rden = 1/(sum_cls pn + NC*eps)  == (numers+eps)/sum(numers+eps)
  - PE warm-up MMs at t=0 (HAM clock gate: PE runs 1.2GHz until ~3.4us of
    sustained activity; warm-up burns the cold window during the input DMA)
  - startup: input DMAs split across the Sync and Scalar issue queues
    (each gets its own 16-engine DMA set -> 2x streaming BW), and chunk 0
    runs k-major in two 4-dt passes so the PE consumes W k-groups as they
    arrive instead of needing all of W for its first dt tile
All transcendentals use only Ln/Exp (one ACT table set, no reloads).
"""

import numpy as np
import ml_dtypes

B, T, NC = 128, 128, 10
CHW, D = 3072, 1024
NCORES = 8
BL = B // NCORES          # 16 sequences per core
TOK = BL * T              # 2048 tokens per core
KT = CHW // 128           # 24 contraction tiles
DT = D // 128             # 8 feature tiles
NCHUNK = 4                # token chunks per core
CH = TOK // NCHUNK        # 512 tokens per chunk
BPC = BL // NCHUNK        # 4 sequences per chunk

EPS_NUMER = 1e-8
EPS_D2 = 1e-12

NUMERS_MODE = "bf16x1"    # single bf16 numers MM (max rel err ~8.6e-3)
WARMUP_MMS = 44           # PE warm-up matmuls issued before the real work
FILLER_MMS = 8            # HAM-keepalive MMs before each chunk-0 k-group

_NC_CACHE = {}
LAST_RESULTS = None       # BassKernelResults of the most recent run (for test.py)


def _build_bass(gamma_is_one):
    import concourse.mybir as mybir
    import concourse.tile as tile
    from concourse import bacc

    f32 = mybir.dt.float32
    bf16 = mybir.dt.bfloat16
    fp8 = mybir.dt.float8e4
    AF = mybir.ActivationFunctionType
    OP = mybir.AluOpType
    PM = mybir.MatmulPerfMode

    # The ACT table-set chooser picks the FIRST set containing each function:
    # keep Ln/Exp (and the Copy/Square/etc that set already contains) pinned
    # to natural_log_exp_and_others so the scalar engine never reloads
    # tables (~1.3us per reload).
    if not getattr(bacc, "_ln_exp_tables_patched", False):
        orig_tables = bacc.get_activation_tables

        def _patched_tables(arch):
            out = {}
            for name, funcs in orig_tables(arch).items():
                if name != "natural_log_exp_and_others":
                    funcs = funcs - {AF.Ln, AF.Exp}
                out[name] = funcs
            return out

        bacc.get_activation_tables = _patched_tables
        bacc._ln_exp_tables_patched = True

    nc = bacc.Bacc("TRN2", target_bir_lowering=False)

    xT_h = nc.dram_tensor("xT", [CHW, TOK], fp8, kind="ExternalInput")
    W_h = nc.dram_tensor("Wt", [CHW, D], fp8, kind="ExternalInput")
    bfe_h = nc.dram_tensor("bfe", [D], f32, kind="ExternalInput")
    TDT = bf16 if NUMERS_MODE == "bf16x1" else f32
    teach_h = nc.dram_tensor("teach", [BL, T, NC], TDT, kind="ExternalInput")
    teach_r = teach_h.rearrange("b s c -> s b c")          # [128, 16, 10]
    negc_h = nc.dram_tensor("negc", [128, 1], f32, kind="ExternalInput")
    gam_h = nc.dram_tensor("gam", [128, 1], f32, kind="ExternalInput")
    y_h = nc.dram_tensor("y", [BL, T, NC], f32, kind="ExternalOutput")

    xT_r = xT_h.rearrange("(kt p) n -> p kt n", p=128)     # [128, 24, 2048]
    W_r = W_h.rearrange("(kt p) d -> p kt d", p=128)       # [128, 24, 1024]
    bfe_r = bfe_h.rearrange("(dt p) -> p dt", p=128)       # [128, 8]

    with tile.TileContext(nc) as tc:
        with (
            tc.tile_pool(name="cpool", bufs=1) as cpool,
            tc.tile_pool(name="xpool", bufs=2) as xpool,
            tc.tile_pool(name="f2pool", bufs=8) as f2pool,
            tc.tile_pool(name="wpool", bufs=3) as wpool,
            tc.tile_pool(name="spool", bufs=3) as spool,
            tc.tile_pool(name="pfpool", bufs=4, space="PSUM") as pfpool,
            tc.tile_pool(name="psqpool", bufs=1, space="PSUM") as psqpool,
            tc.tile_pool(name="pgpool", bufs=2, space="PSUM") as pgpool,
            tc.tile_pool(name="pnpool", bufs=1, space="PSUM") as pnpool,
        ):
            # ---- persistent tiles -------------------------------------
            W_sb = cpool.tile([128, KT, D], fp8, name="W_sb")
            teach_sb = cpool.tile([128, BL, NC], TDT, name="teach_sb")
            bfe_sb = cpool.tile([128, DT], f32, name="bfe_sb")
            negc_sb = cpool.tile([128, 1], f32, name="negc_sb")
            gam_sb = cpool.tile([128, 1], f32, name="gam_sb")
            eps_sb = cpool.tile([128, 1], f32, name="eps_sb")
            ones_sb = cpool.tile([128, 1], bf16, name="ones_sb")
            # rank-2 aug operands for the -0.5*(sq_s + sq_t) update:
            # pg += aug1^T @ aug2 with aug1 = [sqn; ones], aug2 = [ones; sqn]
            # adds sqn[s] + sqn[t] in ONE K=2 matmul.  Engines may only
            # write partition ranges starting at 0, so aug2's sqn row
            # (partition 1) is filled by a tiny SBUF->SBUF DMA per chunk.
            aug1 = cpool.tile([2, TOK], bf16, name="aug1")
            aug2 = cpool.tile([2, TOK], bf16, name="aug2")
            warm_sb = cpool.tile([128, 128], bf16, name="warm_sb")
            fT = [
                cpool.tile([128, TOK], bf16, name=f"fT{i}") for i in range(DT)
            ]

            # ---- PE warm-up: trip the HAM activity window while the input
            # DMAs stream in, so the real feats MMs run at 2.4GHz not 1.2.
            nc.vector.memset(warm_sb, 0.0)
            pwarm = pgpool.tile([128, BPC, 128], f32, name="pg")

            def emit_warmup(count):
                for _ in range(count):
                    nc.tensor.matmul(
                        pwarm[:, 0, :], warm_sb, warm_sb,
                        start=True, stop=True,
                    )

            emit_warmup(WARMUP_MMS)

            # ---- startup DMAs: inputs split across the Sync and Scalar
            # issue queues (separate 16-engine DMA sets stream concurrently),
            # interleaved by k-group and balanced by bytes so chunk-0's
            # k-major consumption matches the arrival order.
            xc0 = xpool.tile([128, KT, CH], fp8, name="xc")
            nc.sync.dma_start(out=W_sb[:, 0:2, :], in_=W_r[:, 0:2, :])
            nc.scalar.dma_start(out=xc0[:, 0:2, :], in_=xT_r[:, 0:2, 0:CH])
            nc.sync.dma_start(out=W_sb[:, 2:8, :], in_=W_r[:, 2:8, :])
            nc.scalar.dma_start(out=xc0[:, 2:8, :], in_=xT_r[:, 2:8, 0:CH])
            nc.scalar.dma_start(out=W_sb[:, 8:16, :], in_=W_r[:, 8:16, :])
            nc.sync.dma_start(out=xc0[:, 8:16, :], in_=xT_r[:, 8:16, 0:CH])
            nc.sync.dma_start(out=W_sb[:, 16:24, :], in_=W_r[:, 16:24, :])
            nc.scalar.dma_start(out=xc0[:, 16:24, :], in_=xT_r[:, 16:24, 0:CH])
            nc.scalar.dma_start(out=teach_sb, in_=teach_r)
            nc.scalar.dma_start(out=bfe_sb, in_=bfe_r)
            nc.scalar.dma_start(out=negc_sb, in_=negc_h[:, :])
            nc.scalar.dma_start(out=gam_sb, in_=gam_h[:, :])

            nc.vector.memset(ones_sb, 1.0)
            nc.vector.memset(eps_sb, EPS_NUMER)
            nc.vector.memset(aug1, 1.0)
            nc.vector.memset(aug2, 1.0)
            # preload both ACT table-set slots during the DMA wait (the
            # lazy load costs ~1.3us on the first Square / first Ln)
            dume = wpool.tile([128, 1], f32, name="dume")
            nc.scalar.activation(dume, eps_sb, AF.Square)
            duml = wpool.tile([128, 1], f32, name="dume2")
            nc.scalar.activation(duml, eps_sb, AF.Exp)

            def evac_dt(c, dt_i, pf, psq):
                # two independent readers of pf:
                #  - DVE evacuates feats (bias add, bf16)
                #  - ACT squares (f+b)^2 for the sq reduction (Square is
                #    in the Ln/Exp table set -> no table reload)
                csl = slice(c * CH, (c + 1) * CH)
                fsl = fT[dt_i][:, csl]
                nc.vector.tensor_scalar(
                    fsl, pf, bfe_sb[:, dt_i:dt_i + 1], None, op0=OP.add,
                )
                f2 = f2pool.tile([128, CH], bf16, name="f2")
                nc.scalar.activation(
                    f2, pf, AF.Square, bias=bfe_sb[:, dt_i:dt_i + 1],
                )
                nc.tensor.matmul(
                    psq, ones_sb, f2,
                    start=(dt_i == 0), stop=(dt_i == DT - 1),
                )

            def emit_sqn(c, psq):
                # sqn = -0.5*sq as a plain bf16 row
                # (|sqn|~500, bf16 rel 2^-9 -> d2 rel err ~5e-4: fine)
                csl = slice(c * CH, (c + 1) * CH)
                nc.vector.tensor_scalar(
                    aug1[0:1, csl], psq, -0.5, None, op0=OP.mult,
                )
                # gpsimd queue: a waiting dma_start blocks its sequencer,
                # and gpsimd has nothing time-critical queued behind it
                nc.gpsimd.dma_start(
                    out=aug2[1:2, csl], in_=aug1[0:1, csl],
                )

            def emit_feats(c, xc, mid=None):
                # mid() is emitted after the 4th dt tile: the previous
                # chunk's epilogue goes there so its ACT/DVE chains start
                # ~half a chunk earlier (and the tail only holds the last
                # chunk's own epilogue).
                psq = psqpool.tile([1, CH], f32, name="psq")
                for dt_i in range(DT):
                    dsl = slice(dt_i * 128, (dt_i + 1) * 128)
                    pf = pfpool.tile([128, CH], f32, name="pf")
                    # fp8 DoubleRow: one MM contracts two adjacent k-tiles
                    for k in range(0, KT, 2):
                        nc.tensor.matmul(
                            pf, W_sb[:, k:k + 2, dsl], xc[:, k:k + 2, :],
                            start=(k == 0), stop=(k == KT - 2),
                            perf_mode=PM.DoubleRow,
                        )
                    evac_dt(c, dt_i, pf, psq)
                    if dt_i == 3 and mid is not None:
                        mid()
                emit_sqn(c, psq)

            def emit_feats_kmajor(xc):
                # chunk 0 only: consume W/x k-groups as the startup DMAs
                # land.  Two passes of 4 dt tiles (4 PSUM banks each).
                # Filler warm-up MMs ahead of each k-group keep the HAM
                # activity window busy across the DMA-arrival stalls so the
                # PE clock stays at 2.4GHz.
                psq = psqpool.tile([1, CH], f32, name="psq")
                for half in range(2):
                    dts = list(range(half * 4, half * 4 + 4))
                    pfs = {}
                    for dt_i in dts:
                        pfs[dt_i] = pfpool.tile([128, CH], f32, name="pf")
                    for k in range(0, KT, 2):
                        if half == 0 and k in (2, 8, 16):
                            emit_warmup(FILLER_MMS)
                        for dt_i in dts:
                            dsl = slice(dt_i * 128, (dt_i + 1) * 128)
                            nc.tensor.matmul(
                                pfs[dt_i], W_sb[:, k:k + 2, dsl],
                                xc[:, k:k + 2, :],
                                start=(k == 0), stop=(k == KT - 2),
                                perf_mode=PM.DoubleRow,
                            )
                    for dt_i in dts:
                        evac_dt(0, dt_i, pfs[dt_i], psq)
                emit_sqn(0, psq)

            def emit_epilogue(c, bis, act_score=False):
                b0 = c * BPC
                n = len(bis)
                # pg[:, i, :] = -0.5 * d2 for sequence b0+bis[i]
                pg = pgpool.tile([128, n, 128], f32, name="pg")
                for i, bi in enumerate(bis):
                    tsl = slice((b0 + bi) * T, (b0 + bi + 1) * T)
                    for dt_i in range(DT):
                        nc.tensor.matmul(
                            pg[:, i, :], fT[dt_i][:, tsl], fT[dt_i][:, tsl],
                            start=(dt_i == 0), stop=False,
                        )
                    nc.tensor.matmul(
                        pg[:, i, :], aug1[:, tsl], aug2[:, tsl],
                        start=False, stop=True,
                    )
                # batched epilogue over the whole [128, n*128] group:
                # dist = exp(0.25*ln(-2*psum)) = d2**0.25; sims=exp(-c*dist).
                # Only the (masked-out) diagonal can go NaN; affine_select
                # fill replaces it with 0.
                lt = wpool.tile([128, n, 128], f32, name="lt")
                nc.scalar.activation(lt, pg, AF.Ln, scale=-2.0)
                dist = wpool.tile([128, n, 128], f32, name="dist")
                nc.scalar.activation(dist, lt, AF.Exp, scale=0.25)
                SDT = bf16 if NUMERS_MODE == "bf16x1" else f32
                sims = wpool.tile([128, n, 128], SDT, name="sims")
                nc.scalar.activation(sims, dist, AF.Exp, scale=negc_sb)
                # zero s >= t per 128-block (kills diagonal NaNs too):
                # iota = t - s - 1 >= 0 keeps sims exactly where s < t.
                simsM = wpool.tile([128, n, 128], SDT, name="simsM")
                nc.gpsimd.affine_select(
                    out=simsM, in_=sims,
                    compare_op=OP.is_ge, fill=0.0,
                    base=-1, pattern=[[0, n], [1, 128]],
                    channel_multiplier=-1,
                )
                # numers[t, cls] = sum_s simsM[s,t] * teach[s, cls]
                pn = pnpool.tile([128, n, NC], f32, name="pn")
                for i, bi in enumerate(bis):
                    nc.tensor.matmul(
                        pn[:, i, :], simsM[:, i, :],
                        teach_sb[:, b0 + bi, :],
                        start=True, stop=True,
                    )
                score = spool.tile([128, n, NC], f32, name="score")
                eng = nc.vector
                if gamma_is_one:
                    # score = (pn+eps)/(sum_cls pn + NC*eps): reduce +
                    # fused add-divide, on DVE (or GpSimd for the final
                    # half so the two tail chains run in parallel).
                    # (free-axis tensor_reduce is DVE-only)
                    den = spool.tile([128, n], f32, name="den")
                    for i in range(n):
                        nc.vector.tensor_reduce(
                            den[:, i:i + 1], pn[:, i, :],
                            axis=mybir.AxisListType.X, op=OP.add,
                        )
                    dent = spool.tile([128, n], f32, name="dent")
                    eng.tensor_scalar(
                        dent, den, NC * EPS_NUMER, None, op0=OP.add,
                    )
                    rden = spool.tile([128, n], f32, name="rden")
                    nc.vector.reciprocal(rden, dent)
                    epsr = spool.tile([128, n], f32, name="epsr")
                    eng.tensor_scalar(
                        epsr, rden, EPS_NUMER, None, op0=OP.mult,
                    )
                    for i in range(n):
                        if act_score:
                            # ACT evacuates rden*pn, GpSimd adds eps*rden:
                            # frees DVE so the two tail halves' chains
                            # overlap across engines
                            sc1 = spool.tile([128, NC], f32, name="sc1")
                            nc.scalar.activation(
                                sc1, pn[:, i, :], AF.Copy,
                                scale=rden[:, i:i + 1],
                            )
                            nc.gpsimd.tensor_scalar(
                                score[:, i, :], sc1, epsr[:, i:i + 1],
                                None, op0=OP.add,
                            )
                        else:
                            eng.tensor_scalar(
                                score[:, i, :], pn[:, i, :], rden[:, i:i + 1],
                                epsr[:, i:i + 1], op0=OP.mult, op1=OP.add,
                            )
                else:
                    # tmp = (numers + eps) ** gamma  via exp(gamma * ln(.))
                    l2 = spool.tile([128, n, NC], f32, name="l2")
                    nc.scalar.activation(l2, pn, AF.Ln, bias=eps_sb)
                    tmp = spool.tile([128, n, NC], f32, name="tmp")
                    nc.scalar.activation(tmp, l2, AF.Exp, scale=gam_sb)
                    den = spool.tile([128, n], f32, name="den")
                    for i in range(n):
                        nc.vector.tensor_reduce(
                            den[:, i:i + 1], tmp[:, i, :],
                            axis=mybir.AxisListType.X, op=OP.add,
                        )
                    rden = spool.tile([128, n], f32, name="rden")
                    nc.vector.reciprocal(rden, den)
                    for i in range(n):
                        nc.vector.tensor_scalar(
                            score[:, i, :], tmp[:, i, :], rden[:, i:i + 1],
                            None, op0=OP.mult,
                        )
                if act_score:
                    nc.gpsimd.memset(score[0:1, :, :], EPS_NUMER)
                else:
                    eng.memset(score[0:1, :, :], EPS_NUMER)
                blo, bhi = b0 + bis[0], b0 + bis[-1] + 1
                nc.sync.dma_start(
                    out=y_h[blo:bhi].rearrange("b s c -> s b c"),
                    in_=score,
                )

            # Software pipeline: emit chunk c's per-sequence epilogue AFTER
            # chunk c+1's feats matmuls, so the PE's in-order queue never
            # stalls waiting on the DVE/ACT chains the epilogue MMs consume.
            # The last chunk's epilogue runs in two halves so its engine
            # chains (ACT/GpSimd/DVE) pipeline in the tail.
            xcs = {0: xc0}

            def mid_hook(c):
                # prefetch the next chunk's x BEFORE the epilogue's output
                # DMA so a waiting out-dma can't delay the prefetch issue
                if c + 1 < NCHUNK:
                    t = xpool.tile([128, KT, CH], fp8, name="xc")
                    nsl = slice((c + 1) * CH, (c + 2) * CH)
                    eng = nc.scalar if (c + 1) in (1, 3) else nc.sync
                    eng.dma_start(out=t, in_=xT_r[:, :, nsl])
                    xcs[c + 1] = t
                if c > 0:
                    emit_epilogue(c - 1, [0, 1, 2, 3])

            for c in range(NCHUNK):
                if c == 0:
                    emit_feats_kmajor(xcs[0])
                    mid_hook(0)
                else:
                    emit_feats(c, xcs[c], mid=lambda cc=c: mid_hook(cc))
            emit_epilogue(NCHUNK - 1, [0, 1], act_score=True)
            emit_epilogue(NCHUNK - 1, [2, 3], act_score=True)

    nc.compile()
    return nc


def _get_bass(gamma_is_one=True):
    key = ("nc", gamma_is_one)
    if key not in _NC_CACHE:
        _NC_CACHE[key] = _build_bass(gamma_is_one)
    return _NC_CACHE[key]


def make_in_maps(data_t, teaching_signal_t, W_fe, b_fe, c, gamma):
    """Host-side prep: cast to the matmul dtype, transpose x, shard 8 ways."""
    fp8 = ml_dtypes.float8_e4m3fn
    x = np.asarray(data_t, np.float32).reshape(B * T, CHW)
    xf8 = x.astype(fp8)
    Wf8 = np.asarray(W_fe, np.float32).astype(fp8)
    bfe = np.ascontiguousarray(np.asarray(b_fe, np.float32).reshape(D))
    teach = np.ascontiguousarray(np.asarray(teaching_signal_t, np.float32))
    if NUMERS_MODE == "bf16x1":
        teach = teach.astype(ml_dtypes.bfloat16)
    cval = np.float32(np.asarray(c, np.float32).reshape(-1)[0])
    gval = np.float32(np.asarray(gamma, np.float32).reshape(-1)[0])
    negc = np.full((128, 1), -cval, np.float32)
    gam = np.full((128, 1), gval, np.float32)

    in_maps = []
    for core in range(NCORES):
        rows = slice(core * TOK, (core + 1) * TOK)
        xT_c = np.ascontiguousarray(xf8[rows].T)          # [3072, 2048]
        m = dict(
            xT=xT_c, Wt=Wf8, bfe=bfe,
            teach=np.ascontiguousarray(teach[core * BL:(core + 1) * BL]),
            negc=negc, gam=gam,
        )
        in_maps.append(m)
    return in_maps


def kernel(responses_t, data_t, teaching_signal_t, W_fe, b_fe, c, gamma):
    global LAST_RESULTS
    from concourse.bass_utils import run_bass_kernel_spmd

    gval = float(np.asarray(gamma, np.float32).reshape(-1)[0])
    in_maps = make_in_maps(data_t, teaching_signal_t, W_fe, b_fe, c, gamma)
    nc = _get_bass(gval == 1.0)
    res = run_bass_kernel_spmd(nc, in_maps, core_ids=list(range(NCORES)))
    LAST_RESULTS = res
    y = np.concatenate([r["y"] for r in res.results], axis=0)  # [128,128,10]
    return np.ascontiguousarray(y[:, :, None, :].astype(np.float32))


# revision 38
# speedup vs baseline: 14243.8380x; 14243.8380x over previous
"""Trainium2 Bass kernel for the ExemplarBaseline retrieval-kNN model.

Math (per batch b, fully independent across b):
    f      = data.reshape(B*T, CHW) @ W_fe + b_fe            (feature extract)
    d2     = ||f_s - f_t||^2 ; dist = d2**0.25
    sims   = exp(-c * dist)
    numers = 1e-8 + sum_{s<t} sims[s,t] * teach[s, cls]
    score  = numers**gamma / sum_cls ; score[t=0] = 1e-8

Sharding: data-parallel over the batch dim B (128) across 8 NeuronCores,
16 sequences per core.  Host pre-casts x/W to fp8 and pre-transposes x so
the device only does matmuls + a fused epilogue:

  - feats^T [D, tok] = W^T @ x^T, fp8 DoubleRow MMs (24 K-tiles -> 12 MMs)
  - evac psum->fT (bf16) on DVE with bias add; f2 = Square(psum+bias) on
    ACT (Square lives in the same ACT table set as Ln/Exp -> no reloads)
  - sq[tok] = ones^T @ f2 on PE; sqn = -0.5*sq as a plain bf16 row
  - per 4-seq chunk: pg[:,bi,:] = Gram (8 bf16 MMs) + 2 rank-1 MMs
    (sqn x ones, ones x sqn) adding -0.5*(sq_s+sq_t) => pg = -0.5*d2
  - batched epilogue on the whole [128, 4*128] chunk:
      d2->dist->sims via Ln/Exp/Exp on ACT, causal mask via ONE
      affine_select on GpSimd, numers = ONE bf16 MM per seq,
      gamma==1 fast path (DVE only): score = pn*rden + eps*rden with
      rden = 1/(sum_cls pn + NC*eps)  == (numers+eps)/sum(numers+eps)
  - PE warm-up MMs at t=0 (HAM clock gate: PE runs 1.2GHz until ~3.4us of
    sustained activity; warm-up burns the cold window during the input DMA)
  - startup: input DMAs split across the Sync and Scalar issue queues
    (each gets its own 16-engine DMA set -> 2x streaming BW), and chunk 0
    runs k-major in two 4-dt passes so the PE consumes W k-groups as they
    arrive instead of needing all of W for its first dt tile
All transcendentals use only Ln/Exp (one ACT table set, no reloads).
"""

import numpy as np
import ml_dtypes

B, T, NC = 128, 128, 10
CHW, D = 3072, 1024
NCORES = 8
BL = B // NCORES          # 16 sequences per core
TOK = BL * T              # 2048 tokens per core
KT = CHW // 128           # 24 contraction tiles
DT = D // 128             # 8 feature tiles
NCHUNK = 4                # token chunks per core
CH = TOK // NCHUNK        # 512 tokens per chunk
BPC = BL // NCHUNK        # 4 sequences per chunk

EPS_NUMER = 1e-8
EPS_D2 = 1e-12

NUMERS_MODE = "bf16x1"    # single bf16 numers MM (max rel err ~8.6e-3)
AUG_MERGE = True          # one K=2 aug MM instead of two rank-1 MMs per seq
WARMUP_MMS = 44           # PE warm-up matmuls issued before the real work
FILLER_MMS = 8            # HAM-keepalive MMs before each chunk-0 k-group

_NC_CACHE = {}
LAST_RESULTS = None       # BassKernelResults of the most recent run (for test.py)


def _build_bass(gamma_is_one):
    import concourse.mybir as mybir
    import concourse.tile as tile
    from concourse import bacc

    f32 = mybir.dt.float32
    bf16 = mybir.dt.bfloat16
    fp8 = mybir.dt.float8e4
    AF = mybir.ActivationFunctionType
    OP = mybir.AluOpType
    PM = mybir.MatmulPerfMode

    # The ACT table-set chooser picks the FIRST set containing each function:
    # keep Ln/Exp (and the Copy/Square/etc that set already contains) pinned
    # to natural_log_exp_and_others so the scalar engine never reloads
    # tables (~1.3us per reload).
    if not getattr(bacc, "_ln_exp_tables_patched", False):
        orig_tables = bacc.get_activation_tables

        def _patched_tables(arch):
            out = {}
            for name, funcs in orig_tables(arch).items():
                if name != "natural_log_exp_and_others":
                    funcs = funcs - {AF.Ln, AF.Exp}
                out[name] = funcs
            return out

        bacc.get_activation_tables = _patched_tables
        bacc._ln_exp_tables_patched = True

    nc = bacc.Bacc("TRN2", target_bir_lowering=False)

    xT_h = nc.dram_tensor("xT", [CHW, TOK], fp8, kind="ExternalInput")
    W_h = nc.dram_tensor("Wt", [CHW, D], fp8, kind="ExternalInput")
    bfe_h = nc.dram_tensor("bfe", [D], f32, kind="ExternalInput")
    TDT = bf16 if NUMERS_MODE == "bf16x1" else f32
    teach_h = nc.dram_tensor("teach", [BL, T, NC], TDT, kind="ExternalInput")
    teach_r = teach_h.rearrange("b s c -> s b c")          # [128, 16, 10]
    negc_h = nc.dram_tensor("negc", [128, 1], f32, kind="ExternalInput")
    gam_h = nc.dram_tensor("gam", [128, 1], f32, kind="ExternalInput")
    y_h = nc.dram_tensor("y", [BL, T, NC], f32, kind="ExternalOutput")

    xT_r = xT_h.rearrange("(kt p) n -> p kt n", p=128)     # [128, 24, 2048]
    W_r = W_h.rearrange("(kt p) d -> p kt d", p=128)       # [128, 24, 1024]
    bfe_r = bfe_h.rearrange("(dt p) -> p dt", p=128)       # [128, 8]

    with tile.TileContext(nc) as tc:
        with (
            tc.tile_pool(name="cpool", bufs=1) as cpool,
            tc.tile_pool(name="xpool", bufs=2) as xpool,
            tc.tile_pool(name="f2pool", bufs=8) as f2pool,
            tc.tile_pool(name="wpool", bufs=3) as wpool,
            tc.tile_pool(name="spool", bufs=3) as spool,
            tc.tile_pool(name="pfpool", bufs=4, space="PSUM") as pfpool,
            tc.tile_pool(name="psqpool", bufs=1, space="PSUM") as psqpool,
            tc.tile_pool(name="pgpool", bufs=2, space="PSUM") as pgpool,
            tc.tile_pool(name="pnpool", bufs=1, space="PSUM") as pnpool,
        ):
            # ---- persistent tiles -------------------------------------
            W_sb = cpool.tile([128, KT, D], fp8, name="W_sb")
            teach_sb = cpool.tile([128, BL, NC], TDT, name="teach_sb")
            bfe_sb = cpool.tile([128, DT], f32, name="bfe_sb")
            negc_sb = cpool.tile([128, 1], f32, name="negc_sb")
            gam_sb = cpool.tile([128, 1], f32, name="gam_sb")
            eps_sb = cpool.tile([128, 1], f32, name="eps_sb")
            ones_sb = cpool.tile([128, 1], bf16, name="ones_sb")
            # rank-2 aug operands for the -0.5*(sq_s + sq_t) update:
            # pg += aug1^T @ aug2 with aug1 = [sqn; ones], aug2 = [ones; sqn]
            # adds sqn[s] + sqn[t] in ONE K=2 matmul.  Engines may only
            # write partition ranges starting at 0, so aug2's sqn row
            # (partition 1) is filled by a tiny SBUF->SBUF DMA per chunk.
            aug1 = cpool.tile([2, TOK], bf16, name="aug1")
            aug2 = cpool.tile([2, TOK], bf16, name="aug2")
            warm_sb = cpool.tile([128, 128], bf16, name="warm_sb")
            fT = [
                cpool.tile([128, TOK], bf16, name=f"fT{i}") for i in range(DT)
            ]

            # ---- PE warm-up: trip the HAM activity window while the input
            # DMAs stream in, so the real feats MMs run at 2.4GHz not 1.2.
            nc.vector.memset(warm_sb, 0.0)
            pwarm = pgpool.tile([128, BPC, 128], f32, name="pg")

            def emit_warmup(count):
                for _ in range(count):
                    nc.tensor.matmul(
                        pwarm[:, 0, :], warm_sb, warm_sb,
                        start=True, stop=True,
                    )

            emit_warmup(WARMUP_MMS)

            # ---- startup DMAs: inputs split across the Sync and Scalar
            # issue queues (separate 16-engine DMA sets stream concurrently),
            # interleaved by k-group and balanced by bytes so chunk-0's
            # k-major consumption matches the arrival order.
            xc0 = xpool.tile([128, KT, CH], fp8, name="xc")
            nc.sync.dma_start(out=W_sb[:, 0:2, :], in_=W_r[:, 0:2, :])
            nc.scalar.dma_start(out=xc0[:, 0:2, :], in_=xT_r[:, 0:2, 0:CH])
            nc.sync.dma_start(out=W_sb[:, 2:8, :], in_=W_r[:, 2:8, :])
            nc.scalar.dma_start(out=xc0[:, 2:8, :], in_=xT_r[:, 2:8, 0:CH])
            nc.scalar.dma_start(out=W_sb[:, 8:16, :], in_=W_r[:, 8:16, :])
            nc.sync.dma_start(out=xc0[:, 8:16, :], in_=xT_r[:, 8:16, 0:CH])
            nc.sync.dma_start(out=W_sb[:, 16:24, :], in_=W_r[:, 16:24, :])
            nc.scalar.dma_start(out=xc0[:, 16:24, :], in_=xT_r[:, 16:24, 0:CH])
            nc.scalar.dma_start(out=teach_sb, in_=teach_r)
            nc.scalar.dma_start(out=bfe_sb, in_=bfe_r)
            nc.scalar.dma_start(out=negc_sb, in_=negc_h[:, :])
            nc.scalar.dma_start(out=gam_sb, in_=gam_h[:, :])

            nc.vector.memset(ones_sb, 1.0)
            nc.vector.memset(eps_sb, EPS_NUMER)
            nc.vector.memset(aug1, 1.0)
            nc.vector.memset(aug2, 1.0)
            # preload both ACT table-set slots during the DMA wait (the
            # lazy load costs ~1.3us on the first Square / first Ln)
            dume = wpool.tile([128, 1], f32, name="dume")
            nc.scalar.activation(dume, eps_sb, AF.Square)
            duml = wpool.tile([128, 1], f32, name="dume2")
            nc.scalar.activation(duml, eps_sb, AF.Exp)

            def evac_dt(c, dt_i, pf, psq):
                # two independent readers of pf:
                #  - DVE evacuates feats (bias add, bf16)
                #  - ACT squares (f+b)^2 for the sq reduction (Square is
                #    in the Ln/Exp table set -> no table reload)
                csl = slice(c * CH, (c + 1) * CH)
                fsl = fT[dt_i][:, csl]
                nc.vector.tensor_scalar(
                    fsl, pf, bfe_sb[:, dt_i:dt_i + 1], None, op0=OP.add,
                )
                f2 = f2pool.tile([128, CH], bf16, name="f2")
                nc.scalar.activation(
                    f2, pf, AF.Square, bias=bfe_sb[:, dt_i:dt_i + 1],
                )
                nc.tensor.matmul(
                    psq, ones_sb, f2,
                    start=(dt_i == 0), stop=(dt_i == DT - 1),
                )

            def emit_sqn(c, psq):
                # sqn = -0.5*sq as a plain bf16 row
                # (|sqn|~500, bf16 rel 2^-9 -> d2 rel err ~5e-4: fine)
                csl = slice(c * CH, (c + 1) * CH)
                nc.vector.tensor_scalar(
                    aug1[0:1, csl], psq, -0.5, None, op0=OP.mult,
                )
                if AUG_MERGE:
                    # gpsimd queue: a waiting dma_start blocks its
                    # sequencer, and gpsimd has nothing time-critical
                    # queued behind this
                    nc.gpsimd.dma_start(
                        out=aug2[1:2, csl], in_=aug1[0:1, csl],
                    )

            def emit_feats(c, xc, mid=None):
                # mid() is emitted after the 4th dt tile: the previous
                # chunk's epilogue goes there so its ACT/DVE chains start
                # ~half a chunk earlier (and the tail only holds the last
                # chunk's own epilogue).
                psq = psqpool.tile([1, CH], f32, name="psq")
                for dt_i in range(DT):
                    dsl = slice(dt_i * 128, (dt_i + 1) * 128)
                    pf = pfpool.tile([128, CH], f32, name="pf")
                    # fp8 DoubleRow: one MM contracts two adjacent k-tiles
                    for k in range(0, KT, 2):
                        nc.tensor.matmul(
                            pf, W_sb[:, k:k + 2, dsl], xc[:, k:k + 2, :],
                            start=(k == 0), stop=(k == KT - 2),
                            perf_mode=PM.DoubleRow,
                        )
                    evac_dt(c, dt_i, pf, psq)
                    if dt_i == 3 and mid is not None:
                        mid()
                emit_sqn(c, psq)

            def emit_feats_kmajor(xc):
                # chunk 0 only: consume W/x k-groups as the startup DMAs
                # land.  Two passes of 4 dt tiles (4 PSUM banks each).
                # Filler warm-up MMs ahead of each k-group keep the HAM
                # activity window busy across the DMA-arrival stalls so the
                # PE clock stays at 2.4GHz.
                psq = psqpool.tile([1, CH], f32, name="psq")
                for half in range(2):
                    dts = list(range(half * 4, half * 4 + 4))
                    pfs = {}
                    for dt_i in dts:
                        pfs[dt_i] = pfpool.tile([128, CH], f32, name="pf")
                    for k in range(0, KT, 2):
                        if half == 0 and k in (2, 8, 16):
                            emit_warmup(FILLER_MMS)
                        for dt_i in dts:
                            dsl = slice(dt_i * 128, (dt_i + 1) * 128)
                            nc.tensor.matmul(
                                pfs[dt_i], W_sb[:, k:k + 2, dsl],
                                xc[:, k:k + 2, :],
                                start=(k == 0), stop=(k == KT - 2),
                                perf_mode=PM.DoubleRow,
                            )
                    for dt_i in dts:
                        evac_dt(0, dt_i, pfs[dt_i], psq)
                emit_sqn(0, psq)

            def emit_epilogue(c, bis, act_score=False):
                b0 = c * BPC
                n = len(bis)
                # pg[:, i, :] = -0.5 * d2 for sequence b0+bis[i]
                pg = pgpool.tile([128, n, 128], f32, name="pg")
                for i, bi in enumerate(bis):
                    tsl = slice((b0 + bi) * T, (b0 + bi + 1) * T)
                    for dt_i in range(DT):
                        nc.tensor.matmul(
                            pg[:, i, :], fT[dt_i][:, tsl], fT[dt_i][:, tsl],
                            start=(dt_i == 0), stop=False,
                        )
                    if AUG_MERGE:
                        nc.tensor.matmul(
                            pg[:, i, :], aug1[:, tsl], aug2[:, tsl],
                            start=False, stop=True,
                        )
                    else:
                        nc.tensor.matmul(
                            pg[:, i, :], aug1[0:1, tsl], aug2[0:1, tsl],
                            start=False, stop=False,
                        )
                        nc.tensor.matmul(
                            pg[:, i, :], aug2[0:1, tsl], aug1[0:1, tsl],
                            start=False, stop=True,
                        )
                # batched epilogue over the whole [128, n*128] group:
                # dist = exp(0.25*ln(-2*psum)) = d2**0.25; sims=exp(-c*dist).
                # Only the (masked-out) diagonal can go NaN; affine_select
                # fill replaces it with 0.
                lt = wpool.tile([128, n, 128], f32, name="lt")
                nc.scalar.activation(lt, pg, AF.Ln, scale=-2.0)
                dist = wpool.tile([128, n, 128], f32, name="dist")
                nc.scalar.activation(dist, lt, AF.Exp, scale=0.25)
                SDT = bf16 if NUMERS_MODE == "bf16x1" else f32
                sims = wpool.tile([128, n, 128], SDT, name="sims")
                nc.scalar.activation(sims, dist, AF.Exp, scale=negc_sb)
                # zero s >= t per 128-block (kills diagonal NaNs too):
                # iota = t - s - 1 >= 0 keeps sims exactly where s < t.
                simsM = wpool.tile([128, n, 128], SDT, name="simsM")
                nc.gpsimd.affine_select(
                    out=simsM, in_=sims,
                    compare_op=OP.is_ge, fill=0.0,
                    base=-1, pattern=[[0, n], [1, 128]],
                    channel_multiplier=-1,
                )
                # numers[t, cls] = sum_s simsM[s,t] * teach[s, cls]
                pn = pnpool.tile([128, n, NC], f32, name="pn")
                for i, bi in enumerate(bis):
                    nc.tensor.matmul(
                        pn[:, i, :], simsM[:, i, :],
                        teach_sb[:, b0 + bi, :],
                        start=True, stop=True,
                    )
                score = spool.tile([128, n, NC], f32, name="score")
                eng = nc.vector
                if gamma_is_one:
                    # score = (pn+eps)/(sum_cls pn + NC*eps): reduce +
                    # fused add-divide, on DVE (or GpSimd for the final
                    # half so the two tail chains run in parallel).
                    # (free-axis tensor_reduce is DVE-only)
                    den = spool.tile([128, n], f32, name="den")
                    for i in range(n):
                        nc.vector.tensor_reduce(
                            den[:, i:i + 1], pn[:, i, :],
                            axis=mybir.AxisListType.X, op=OP.add,
                        )
                    dent = spool.tile([128, n], f32, name="dent")
                    eng.tensor_scalar(
                        dent, den, NC * EPS_NUMER, None, op0=OP.add,
                    )
                    rden = spool.tile([128, n], f32, name="rden")
                    nc.vector.reciprocal(rden, dent)
                    epsr = spool.tile([128, n], f32, name="epsr")
                    eng.tensor_scalar(
                        epsr, rden, EPS_NUMER, None, op0=OP.mult,
                    )
                    for i in range(n):
                        if act_score:
                            # ACT evacuates rden*pn, GpSimd adds eps*rden:
                            # frees DVE so the two tail halves' chains
                            # overlap across engines
                            sc1 = spool.tile([128, NC], f32, name="sc1")
                            nc.scalar.activation(
                                sc1, pn[:, i, :], AF.Copy,
                                scale=rden[:, i:i + 1],
                            )
                            nc.gpsimd.tensor_scalar(
                                score[:, i, :], sc1, epsr[:, i:i + 1],
                                None, op0=OP.add,
                            )
                        else:
                            eng.tensor_scalar(
                                score[:, i, :], pn[:, i, :], rden[:, i:i + 1],
                                epsr[:, i:i + 1], op0=OP.mult, op1=OP.add,
                            )
                else:
                    # tmp = (numers + eps) ** gamma  via exp(gamma * ln(.))
                    l2 = spool.tile([128, n, NC], f32, name="l2")
                    nc.scalar.activation(l2, pn, AF.Ln, bias=eps_sb)
                    tmp = spool.tile([128, n, NC], f32, name="tmp")
                    nc.scalar.activation(tmp, l2, AF.Exp, scale=gam_sb)
                    den = spool.tile([128, n], f32, name="den")
                    for i in range(n):
                        nc.vector.tensor_reduce(
                            den[:, i:i + 1], tmp[:, i, :],
                            axis=mybir.AxisListType.X, op=OP.add,
                        )
                    rden = spool.tile([128, n], f32, name="rden")
                    nc.vector.reciprocal(rden, den)
                    for i in range(n):
                        nc.vector.tensor_scalar(
                            score[:, i, :], tmp[:, i, :], rden[:, i:i + 1],
                            None, op0=OP.mult,
                        )
                if act_score:
                    nc.gpsimd.memset(score[0:1, :, :], EPS_NUMER)
                else:
                    eng.memset(score[0:1, :, :], EPS_NUMER)
                blo, bhi = b0 + bis[0], b0 + bis[-1] + 1
                nc.sync.dma_start(
                    out=y_h[blo:bhi].rearrange("b s c -> s b c"),
                    in_=score,
                )

            # Software pipeline: emit chunk c's per-sequence epilogue AFTER
            # chunk c+1's feats matmuls, so the PE's in-order queue never
            # stalls waiting on the DVE/ACT chains the epilogue MMs consume.
            # The last chunk's epilogue runs in two halves so its engine
            # chains (ACT/GpSimd/DVE) pipeline in the tail.
            xcs = {0: xc0}

            def mid_hook(c):
                # prefetch the next chunk's x BEFORE the epilogue's output
                # DMA so a waiting out-dma can't delay the prefetch issue
                if c + 1 < NCHUNK:
                    t = xpool.tile([128, KT, CH], fp8, name="xc")
                    nsl = slice((c + 1) * CH, (c + 2) * CH)
                    eng = nc.scalar if (c + 1) in (1, 3) else nc.sync
                    eng.dma_start(out=t, in_=xT_r[:, :, nsl])
                    xcs[c + 1] = t
                if c > 0:
                    emit_epilogue(c - 1, [0, 1, 2, 3])

            for c in range(NCHUNK):
                if c == 0:
                    emit_feats_kmajor(xcs[0])
                    mid_hook(0)
                else:
                    emit_feats(c, xcs[c], mid=lambda cc=c: mid_hook(cc))
            emit_epilogue(NCHUNK - 1, [0, 1], act_score=True)
            emit_epilogue(NCHUNK - 1, [2, 3], act_score=True)

    nc.compile()
    return nc


def _get_bass(gamma_is_one=True):
    key = ("nc", gamma_is_one)
    if key not in _NC_CACHE:
        _NC_CACHE[key] = _build_bass(gamma_is_one)
    return _NC_CACHE[key]


def make_in_maps(data_t, teaching_signal_t, W_fe, b_fe, c, gamma):
    """Host-side prep: cast to the matmul dtype, transpose x, shard 8 ways."""
    fp8 = ml_dtypes.float8_e4m3fn
    x = np.asarray(data_t, np.float32).reshape(B * T, CHW)
    xf8 = x.astype(fp8)
    Wf8 = np.asarray(W_fe, np.float32).astype(fp8)
    bfe = np.ascontiguousarray(np.asarray(b_fe, np.float32).reshape(D))
    teach = np.ascontiguousarray(np.asarray(teaching_signal_t, np.float32))
    if NUMERS_MODE == "bf16x1":
        teach = teach.astype(ml_dtypes.bfloat16)
    cval = np.float32(np.asarray(c, np.float32).reshape(-1)[0])
    gval = np.float32(np.asarray(gamma, np.float32).reshape(-1)[0])
    negc = np.full((128, 1), -cval, np.float32)
    gam = np.full((128, 1), gval, np.float32)

    in_maps = []
    for core in range(NCORES):
        rows = slice(core * TOK, (core + 1) * TOK)
        xT_c = np.ascontiguousarray(xf8[rows].T)          # [3072, 2048]
        m = dict(
            xT=xT_c, Wt=Wf8, bfe=bfe,
            teach=np.ascontiguousarray(teach[core * BL:(core + 1) * BL]),
            negc=negc, gam=gam,
        )
        in_maps.append(m)
    return in_maps


def kernel(responses_t, data_t, teaching_signal_t, W_fe, b_fe, c, gamma):
    global LAST_RESULTS
    from concourse.bass_utils import run_bass_kernel_spmd

    gval = float(np.asarray(gamma, np.float32).reshape(-1)[0])
    in_maps = make_in_maps(data_t, teaching_signal_t, W_fe, b_fe, c, gamma)
    nc = _get_bass(gval == 1.0)
    res = run_bass_kernel_spmd(nc, in_maps, core_ids=list(range(NCORES)))
    LAST_RESULTS = res
    y = np.concatenate([r["y"] for r in res.results], axis=0)  # [128,128,10]
    return np.ascontiguousarray(y[:, :, None, :].astype(np.float32))


# revision 39
# speedup vs baseline: 14716.0572x; 1.0332x over previous
"""Trainium2 Bass kernel for the ExemplarBaseline retrieval-kNN model.

Math (per batch b, fully independent across b):
    f      = data.reshape(B*T, CHW) @ W_fe + b_fe            (feature extract)
    d2     = ||f_s - f_t||^2 ; dist = d2**0.25
    sims   = exp(-c * dist)
    numers = 1e-8 + sum_{s<t} sims[s,t] * teach[s, cls]
    score  = numers**gamma / sum_cls ; score[t=0] = 1e-8

Sharding: data-parallel over the batch dim B (128) across 8 NeuronCores,
16 sequences per core.  Host pre-casts x/W to fp8 and pre-transposes x so
the device only does matmuls + a fused epilogue:

  - feats^T [D, tok] = W^T @ x^T, fp8 DoubleRow MMs (24 K-tiles -> 12 MMs)
  - evac psum->fT (bf16) on DVE with bias add; f2 = Square(psum+bias) on
    ACT (Square lives in the same ACT table set as Ln/Exp -> no reloads)
  - sq[tok] = ones^T @ f2 on PE; sqn = -0.5*sq as a plain bf16 row
  - per 4-seq chunk: pg[:,bi,:] = Gram (8 bf16 MMs) + 2 rank-1 MMs
    (sqn x ones, ones x sqn) adding -0.5*(sq_s+sq_t) => pg = -0.5*d2
  - batched epilogue on the whole [128, 4*128] chunk:
      d2->dist->sims via Ln/Exp/Exp on ACT, causal mask via ONE
      affine_select on GpSimd, numers = ONE bf16 MM per seq,
      gamma==1 fast path (DVE only): score = pn*rden + eps*rden with
      rden = 1/(sum_cls pn + NC*eps)  == (numers+eps)/sum(numers+eps)
  - PE warm-up MMs at t=0 (HAM clock gate: PE runs 1.2GHz until ~3.4us of
    sustained activity; warm-up burns the cold window during the input DMA)
  - startup: input DMAs split across the Sync and Scalar issue queues
    (each gets its own 16-engine DMA set -> 2x streaming BW), and chunk 0
    runs k-major in two 4-dt passes so the PE consumes W k-groups as they
    arrive instead of needing all of W for its first dt tile
All transcendentals use only Ln/Exp (one ACT table set, no reloads).
"""

import numpy as np
import ml_dtypes

B, T, NC = 128, 128, 10
CHW, D = 3072, 1024
NCORES = 8
BL = B // NCORES          # 16 sequences per core
TOK = BL * T              # 2048 tokens per core
KT = CHW // 128           # 24 contraction tiles
DT = D // 128             # 8 feature tiles
NCHUNK = 4                # token chunks per core
CH = TOK // NCHUNK        # 512 tokens per chunk
BPC = BL // NCHUNK        # 4 sequences per chunk

EPS_NUMER = 1e-8
EPS_D2 = 1e-12

NUMERS_MODE = "bf16x1"    # single bf16 numers MM (max rel err ~8.6e-3)
AUG_MERGE = False          # one K=2 aug MM instead of two rank-1 MMs per seq
WARMUP_MMS = 44           # PE warm-up matmuls issued before the real work
FILLER_MMS = 8            # HAM-keepalive MMs before each chunk-0 k-group

_NC_CACHE = {}
LAST_RESULTS = None       # BassKernelResults of the most recent run (for test.py)


def _build_bass(gamma_is_one):
    import concourse.mybir as mybir
    import concourse.tile as tile
    from concourse import bacc

    f32 = mybir.dt.float32
    bf16 = mybir.dt.bfloat16
    fp8 = mybir.dt.float8e4
    AF = mybir.ActivationFunctionType
    OP = mybir.AluOpType
    PM = mybir.MatmulPerfMode

    # The ACT table-set chooser picks the FIRST set containing each function:
    # keep Ln/Exp (and the Copy/Square/etc that set already contains) pinned
    # to natural_log_exp_and_others so the scalar engine never reloads
    # tables (~1.3us per reload).
    if not getattr(bacc, "_ln_exp_tables_patched", False):
        orig_tables = bacc.get_activation_tables

        def _patched_tables(arch):
            out = {}
            for name, funcs in orig_tables(arch).items():
                if name != "natural_log_exp_and_others":
                    funcs = funcs - {AF.Ln, AF.Exp}
                out[name] = funcs
            return out

        bacc.get_activation_tables = _patched_tables
        bacc._ln_exp_tables_patched = True

    nc = bacc.Bacc("TRN2", target_bir_lowering=False)

    xT_h = nc.dram_tensor("xT", [CHW, TOK], fp8, kind="ExternalInput")
    W_h = nc.dram_tensor("Wt", [CHW, D], fp8, kind="ExternalInput")
    bfe_h = nc.dram_tensor("bfe", [D], f32, kind="ExternalInput")
    TDT = bf16 if NUMERS_MODE == "bf16x1" else f32
    teach_h = nc.dram_tensor("teach", [BL, T, NC], TDT, kind="ExternalInput")
    teach_r = teach_h.rearrange("b s c -> s b c")          # [128, 16, 10]
    negc_h = nc.dram_tensor("negc", [128, 1], f32, kind="ExternalInput")
    gam_h = nc.dram_tensor("gam", [128, 1], f32, kind="ExternalInput")
    y_h = nc.dram_tensor("y", [BL, T, NC], f32, kind="ExternalOutput")

    xT_r = xT_h.rearrange("(kt p) n -> p kt n", p=128)     # [128, 24, 2048]
    W_r = W_h.rearrange("(kt p) d -> p kt d", p=128)       # [128, 24, 1024]
    bfe_r = bfe_h.rearrange("(dt p) -> p dt", p=128)       # [128, 8]

    with tile.TileContext(nc) as tc:
        with (
            tc.tile_pool(name="cpool", bufs=1) as cpool,
            tc.tile_pool(name="xpool", bufs=2) as xpool,
            tc.tile_pool(name="f2pool", bufs=8) as f2pool,
            tc.tile_pool(name="wpool", bufs=3) as wpool,
            tc.tile_pool(name="spool", bufs=3) as spool,
            tc.tile_pool(name="pfpool", bufs=4, space="PSUM") as pfpool,
            tc.tile_pool(name="psqpool", bufs=1, space="PSUM") as psqpool,
            tc.tile_pool(name="pgpool", bufs=2, space="PSUM") as pgpool,
            tc.tile_pool(name="pnpool", bufs=1, space="PSUM") as pnpool,
        ):
            # ---- persistent tiles -------------------------------------
            W_sb = cpool.tile([128, KT, D], fp8, name="W_sb")
            teach_sb = cpool.tile([128, BL, NC], TDT, name="teach_sb")
            bfe_sb = cpool.tile([128, DT], f32, name="bfe_sb")
            negc_sb = cpool.tile([128, 1], f32, name="negc_sb")
            gam_sb = cpool.tile([128, 1], f32, name="gam_sb")
            eps_sb = cpool.tile([128, 1], f32, name="eps_sb")
            ones_sb = cpool.tile([128, 1], bf16, name="ones_sb")
            # rank-2 aug operands for the -0.5*(sq_s + sq_t) update:
            # pg += aug1^T @ aug2 with aug1 = [sqn; ones], aug2 = [ones; sqn]
            # adds sqn[s] + sqn[t] in ONE K=2 matmul.  Engines may only
            # write partition ranges starting at 0, so aug2's sqn row
            # (partition 1) is filled by a tiny SBUF->SBUF DMA per chunk.
            aug1 = cpool.tile([2, TOK], bf16, name="aug1")
            aug2 = cpool.tile([2, TOK], bf16, name="aug2")
            warm_sb = cpool.tile([128, 128], bf16, name="warm_sb")
            fT = [
                cpool.tile([128, TOK], bf16, name=f"fT{i}") for i in range(DT)
            ]

            # ---- PE warm-up: trip the HAM activity window while the input
            # DMAs stream in, so the real feats MMs run at 2.4GHz not 1.2.
            nc.vector.memset(warm_sb, 0.0)
            pwarm = pgpool.tile([128, BPC, 128], f32, name="pg")

            def emit_warmup(count):
                for _ in range(count):
                    nc.tensor.matmul(
                        pwarm[:, 0, :], warm_sb, warm_sb,
                        start=True, stop=True,
                    )

            emit_warmup(WARMUP_MMS)

            # ---- startup DMAs: inputs split across the Sync and Scalar
            # issue queues (separate 16-engine DMA sets stream concurrently),
            # interleaved by k-group and balanced by bytes so chunk-0's
            # k-major consumption matches the arrival order.
            xc0 = xpool.tile([128, KT, CH], fp8, name="xc")
            nc.sync.dma_start(out=W_sb[:, 0:2, :], in_=W_r[:, 0:2, :])
            nc.scalar.dma_start(out=xc0[:, 0:2, :], in_=xT_r[:, 0:2, 0:CH])
            nc.sync.dma_start(out=W_sb[:, 2:8, :], in_=W_r[:, 2:8, :])
            nc.scalar.dma_start(out=xc0[:, 2:8, :], in_=xT_r[:, 2:8, 0:CH])
            nc.scalar.dma_start(out=W_sb[:, 8:16, :], in_=W_r[:, 8:16, :])
            nc.sync.dma_start(out=xc0[:, 8:16, :], in_=xT_r[:, 8:16, 0:CH])
            nc.sync.dma_start(out=W_sb[:, 16:24, :], in_=W_r[:, 16:24, :])
            nc.scalar.dma_start(out=xc0[:, 16:24, :], in_=xT_r[:, 16:24, 0:CH])
            nc.scalar.dma_start(out=teach_sb, in_=teach_r)
            nc.scalar.dma_start(out=bfe_sb, in_=bfe_r)
            nc.scalar.dma_start(out=negc_sb, in_=negc_h[:, :])
            nc.scalar.dma_start(out=gam_sb, in_=gam_h[:, :])

            nc.vector.memset(ones_sb, 1.0)
            nc.vector.memset(eps_sb, EPS_NUMER)
            nc.vector.memset(aug1, 1.0)
            nc.vector.memset(aug2, 1.0)
            # preload both ACT table-set slots during the DMA wait (the
            # lazy load costs ~1.3us on the first Square / first Ln)
            dume = wpool.tile([128, 1], f32, name="dume")
            nc.scalar.activation(dume, eps_sb, AF.Square)
            duml = wpool.tile([128, 1], f32, name="dume2")
            nc.scalar.activation(duml, eps_sb, AF.Exp)

            def evac_dt(c, dt_i, pf, psq):
                # two independent readers of pf:
                #  - DVE evacuates feats (bias add, bf16)
                #  - ACT squares (f+b)^2 for the sq reduction (Square is
                #    in the Ln/Exp table set -> no table reload)
                csl = slice(c * CH, (c + 1) * CH)
                fsl = fT[dt_i][:, csl]
                nc.vector.tensor_scalar(
                    fsl, pf, bfe_sb[:, dt_i:dt_i + 1], None, op0=OP.add,
                )
                f2 = f2pool.tile([128, CH], bf16, name="f2")
                nc.scalar.activation(
                    f2, pf, AF.Square, bias=bfe_sb[:, dt_i:dt_i + 1],
                )
                nc.tensor.matmul(
                    psq, ones_sb, f2,
                    start=(dt_i == 0), stop=(dt_i == DT - 1),
                )

            def emit_sqn(c, psq):
                # sqn = -0.5*sq as a plain bf16 row
                # (|sqn|~500, bf16 rel 2^-9 -> d2 rel err ~5e-4: fine)
                csl = slice(c * CH, (c + 1) * CH)
                nc.vector.tensor_scalar(
                    aug1[0:1, csl], psq, -0.5, None, op0=OP.mult,
                )
                if AUG_MERGE:
                    # gpsimd queue: a waiting dma_start blocks its
                    # sequencer, and gpsimd has nothing time-critical
                    # queued behind this
                    nc.gpsimd.dma_start(
                        out=aug2[1:2, csl], in_=aug1[0:1, csl],
                    )

            def emit_feats(c, xc, mid=None):
                # mid() is emitted after the 4th dt tile: the previous
                # chunk's epilogue goes there so its ACT/DVE chains start
                # ~half a chunk earlier (and the tail only holds the last
                # chunk's own epilogue).
                psq = psqpool.tile([1, CH], f32, name="psq")
                for dt_i in range(DT):
                    dsl = slice(dt_i * 128, (dt_i + 1) * 128)
                    pf = pfpool.tile([128, CH], f32, name="pf")
                    # fp8 DoubleRow: one MM contracts two adjacent k-tiles
                    for k in range(0, KT, 2):
                        nc.tensor.matmul(
                            pf, W_sb[:, k:k + 2, dsl], xc[:, k:k + 2, :],
                            start=(k == 0), stop=(k == KT - 2),
                            perf_mode=PM.DoubleRow,
                        )
                    evac_dt(c, dt_i, pf, psq)
                    if dt_i == 3 and mid is not None:
                        mid()
                emit_sqn(c, psq)

            def emit_feats_kmajor(xc):
                # chunk 0 only: consume W/x k-groups as the startup DMAs
                # land.  Two passes of 4 dt tiles (4 PSUM banks each).
                # Filler warm-up MMs ahead of each k-group keep the HAM
                # activity window busy across the DMA-arrival stalls so the
                # PE clock stays at 2.4GHz.
                psq = psqpool.tile([1, CH], f32, name="psq")
                for half in range(2):
                    dts = list(range(half * 4, half * 4 + 4))
                    pfs = {}
                    for dt_i in dts:
                        pfs[dt_i] = pfpool.tile([128, CH], f32, name="pf")
                    for k in range(0, KT, 2):
                        if half == 0 and k in (2, 8, 16):
                            emit_warmup(FILLER_MMS)
                        for dt_i in dts:
                            dsl = slice(dt_i * 128, (dt_i + 1) * 128)
                            nc.tensor.matmul(
                                pfs[dt_i], W_sb[:, k:k + 2, dsl],
                                xc[:, k:k + 2, :],
                                start=(k == 0), stop=(k == KT - 2),
                                perf_mode=PM.DoubleRow,
                            )
                    for dt_i in dts:
                        evac_dt(0, dt_i, pfs[dt_i], psq)
                emit_sqn(0, psq)

            def emit_epilogue(c, bis, act_score=False):
                b0 = c * BPC
                n = len(bis)
                # pg[:, i, :] = -0.5 * d2 for sequence b0+bis[i]
                pg = pgpool.tile([128, n, 128], f32, name="pg")
                for i, bi in enumerate(bis):
                    tsl = slice((b0 + bi) * T, (b0 + bi + 1) * T)
                    for dt_i in range(DT):
                        nc.tensor.matmul(
                            pg[:, i, :], fT[dt_i][:, tsl], fT[dt_i][:, tsl],
                            start=(dt_i == 0), stop=False,
                        )
                    if AUG_MERGE:
                        nc.tensor.matmul(
                            pg[:, i, :], aug1[:, tsl], aug2[:, tsl],
                            start=False, stop=True,
                        )
                    else:
                        nc.tensor.matmul(
                            pg[:, i, :], aug1[0:1, tsl], aug2[0:1, tsl],
                            start=False, stop=False,
                        )
                        nc.tensor.matmul(
                            pg[:, i, :], aug2[0:1, tsl], aug1[0:1, tsl],
                            start=False, stop=True,
                        )
                # batched epilogue over the whole [128, n*128] group:
                # dist = exp(0.25*ln(-2*psum)) = d2**0.25; sims=exp(-c*dist).
                # Only the (masked-out) diagonal can go NaN; affine_select
                # fill replaces it with 0.
                lt = wpool.tile([128, n, 128], f32, name="lt")
                nc.scalar.activation(lt, pg, AF.Ln, scale=-2.0)
                dist = wpool.tile([128, n, 128], f32, name="dist")
                nc.scalar.activation(dist, lt, AF.Exp, scale=0.25)
                SDT = bf16 if NUMERS_MODE == "bf16x1" else f32
                sims = wpool.tile([128, n, 128], SDT, name="sims")
                nc.scalar.activation(sims, dist, AF.Exp, scale=negc_sb)
                # zero s >= t per 128-block (kills diagonal NaNs too):
                # iota = t - s - 1 >= 0 keeps sims exactly where s < t.
                simsM = wpool.tile([128, n, 128], SDT, name="simsM")
                nc.gpsimd.affine_select(
                    out=simsM, in_=sims,
                    compare_op=OP.is_ge, fill=0.0,
                    base=-1, pattern=[[0, n], [1, 128]],
                    channel_multiplier=-1,
                )
                # numers[t, cls] = sum_s simsM[s,t] * teach[s, cls]
                pn = pnpool.tile([128, n, NC], f32, name="pn")
                for i, bi in enumerate(bis):
                    nc.tensor.matmul(
                        pn[:, i, :], simsM[:, i, :],
                        teach_sb[:, b0 + bi, :],
                        start=True, stop=True,
                    )
                score = spool.tile([128, n, NC], f32, name="score")
                eng = nc.vector
                if gamma_is_one:
                    # score = (pn+eps)/(sum_cls pn + NC*eps): reduce +
                    # fused add-divide, on DVE (or GpSimd for the final
                    # half so the two tail chains run in parallel).
                    # (free-axis tensor_reduce is DVE-only)
                    den = spool.tile([128, n], f32, name="den")
                    for i in range(n):
                        nc.vector.tensor_reduce(
                            den[:, i:i + 1], pn[:, i, :],
                            axis=mybir.AxisListType.X, op=OP.add,
                        )
                    dent = spool.tile([128, n], f32, name="dent")
                    eng.tensor_scalar(
                        dent, den, NC * EPS_NUMER, None, op0=OP.add,
                    )
                    rden = spool.tile([128, n], f32, name="rden")
                    nc.vector.reciprocal(rden, dent)
                    epsr = spool.tile([128, n], f32, name="epsr")
                    eng.tensor_scalar(
                        epsr, rden, EPS_NUMER, None, op0=OP.mult,
                    )
                    for i in range(n):
                        if act_score:
                            # ACT evacuates rden*pn, GpSimd adds eps*rden:
                            # frees DVE so the two tail halves' chains
                            # overlap across engines
                            sc1 = spool.tile([128, NC], f32, name="sc1")
                            nc.scalar.activation(
                                sc1, pn[:, i, :], AF.Copy,
                                scale=rden[:, i:i + 1],
                            )
                            nc.gpsimd.tensor_scalar(
                                score[:, i, :], sc1, epsr[:, i:i + 1],
                                None, op0=OP.add,
                            )
                        else:
                            eng.tensor_scalar(
                                score[:, i, :], pn[:, i, :], rden[:, i:i + 1],
                                epsr[:, i:i + 1], op0=OP.mult, op1=OP.add,
                            )
                else:
                    # tmp = (numers + eps) ** gamma  via exp(gamma * ln(.))
                    l2 = spool.tile([128, n, NC], f32, name="l2")
                    nc.scalar.activation(l2, pn, AF.Ln, bias=eps_sb)
                    tmp = spool.tile([128, n, NC], f32, name="tmp")
                    nc.scalar.activation(tmp, l2, AF.Exp, scale=gam_sb)
                    den = spool.tile([128, n], f32, name="den")
                    for i in range(n):
                        nc.vector.tensor_reduce(
                            den[:, i:i + 1], tmp[:, i, :],
                            axis=mybir.AxisListType.X, op=OP.add,
                        )
                    rden = spool.tile([128, n], f32, name="rden")
                    nc.vector.reciprocal(rden, den)
                    for i in range(n):
                        nc.vector.tensor_scalar(
                            score[:, i, :], tmp[:, i, :], rden[:, i:i + 1],
                            None, op0=OP.mult,
                        )
                if act_score:
                    nc.gpsimd.memset(score[0:1, :, :], EPS_NUMER)
                else:
                    eng.memset(score[0:1, :, :], EPS_NUMER)
                blo, bhi = b0 + bis[0], b0 + bis[-1] + 1
                nc.sync.dma_start(
                    out=y_h[blo:bhi].rearrange("b s c -> s b c"),
                    in_=score,
                )

            # Software pipeline: emit chunk c's per-sequence epilogue AFTER
            # chunk c+1's feats matmuls, so the PE's in-order queue never
            # stalls waiting on the DVE/ACT chains the epilogue MMs consume.
            # The last chunk's epilogue runs in two halves so its engine
            # chains (ACT/GpSimd/DVE) pipeline in the tail.
            xcs = {0: xc0}

            def mid_hook(c):
                # prefetch the next chunk's x BEFORE the epilogue's output
                # DMA so a waiting out-dma can't delay the prefetch issue
                if c + 1 < NCHUNK:
                    t = xpool.tile([128, KT, CH], fp8, name="xc")
                    nsl = slice((c + 1) * CH, (c + 2) * CH)
                    eng = nc.scalar if (c + 1) in (1, 3) else nc.sync
                    eng.dma_start(out=t, in_=xT_r[:, :, nsl])
                    xcs[c + 1] = t
                if c > 0:
                    emit_epilogue(c - 1, [0, 1, 2, 3])

            for c in range(NCHUNK):
                if c == 0:
                    emit_feats_kmajor(xcs[0])
                    mid_hook(0)
                else:
                    emit_feats(c, xcs[c], mid=lambda cc=c: mid_hook(cc))
            emit_epilogue(NCHUNK - 1, [0, 1], act_score=True)
            emit_epilogue(NCHUNK - 1, [2, 3], act_score=True)

    nc.compile()
    return nc


def _get_bass(gamma_is_one=True):
    key = ("nc", gamma_is_one)
    if key not in _NC_CACHE:
        _NC_CACHE[key] = _build_bass(gamma_is_one)
    return _NC_CACHE[key]


def make_in_maps(data_t, teaching_signal_t, W_fe, b_fe, c, gamma):
    """Host-side prep: cast to the matmul dtype, transpose x, shard 8 ways."""
    fp8 = ml_dtypes.float8_e4m3fn
    x = np.asarray(data_t, np.float32).reshape(B * T, CHW)
    xf8 = x.astype(fp8)
    Wf8 = np.asarray(W_fe, np.float32).astype(fp8)
    bfe = np.ascontiguousarray(np.asarray(b_fe, np.float32).reshape(D))
    teach = np.ascontiguousarray(np.asarray(teaching_signal_t, np.float32))
    if NUMERS_MODE == "bf16x1":
        teach = teach.astype(ml_dtypes.bfloat16)
    cval = np.float32(np.asarray(c, np.float32).reshape(-1)[0])
    gval = np.float32(np.asarray(gamma, np.float32).reshape(-1)[0])
    negc = np.full((128, 1), -cval, np.float32)
    gam = np.full((128, 1), gval, np.float32)

    in_maps = []
    for core in range(NCORES):
        rows = slice(core * TOK, (core + 1) * TOK)
        xT_c = np.ascontiguousarray(xf8[rows].T)          # [3072, 2048]
        m = dict(
            xT=xT_c, Wt=Wf8, bfe=bfe,
            teach=np.ascontiguousarray(teach[core * BL:(core + 1) * BL]),
            negc=negc, gam=gam,
        )
        in_maps.append(m)
    return in_maps


def kernel(responses_t, data_t, teaching_signal_t, W_fe, b_fe, c, gamma):
    global LAST_RESULTS
    from concourse.bass_utils import run_bass_kernel_spmd

    gval = float(np.asarray(gamma, np.float32).reshape(-1)[0])
    in_maps = make_in_maps(data_t, teaching_signal_t, W_fe, b_fe, c, gamma)
    nc = _get_bass(gval == 1.0)
    res = run_bass_kernel_spmd(nc, in_maps, core_ids=list(range(NCORES)))
    LAST_RESULTS = res
    y = np.concatenate([r["y"] for r in res.results], axis=0)  # [128,128,10]
    return np.ascontiguousarray(y[:, :, None, :].astype(np.float32))
